# revision 15
# baseline (speedup 1.0000x reference)
"""Trainium2 Bass kernel for a 2-layer GCN (AttributeDecoder):

    out = relu(adj @ relu(adj @ (X @ W1)) @ W2)

with N=8192, D_IN=64, D_HID=128, D_OUT=256, fp32 in/out.

Strategy (8 NeuronCores, SPMD), v2:
  - Row-shard adj across cores: core i owns rows [1024*i, 1024*(i+1)).
  - adj is down-converted to fp8-e4m3 ON HOST (adj entries are U[0,1); the
    propagation sums are sign-coherent after relu, so fp8 adj costs only
    ~3e-3 end-to-end rel err vs the 2e-2 gate). The PE accepts a mixed
    bf16-stationary x fp8-moving matmul (HW-verified exact), so XW1/H1
    stay bf16. adj HBM traffic halves vs bf16: 8MB/core, read ONCE.
  - Slab packing is m-half-major: slab s in [0,8) holds k-blocks 8s..8s+7
    for own-node half m in [0,512); slabs 8..15 the same k-blocks for
    m in [512,1024). Each L1 PSUM-bank pass consumes a contiguous slab
    stream with ZERO bank alternation, and H1's first half is finished
    (and AllGather'd) while the second half still streams.
  - Layer 1: psum_h[h, m] = sum_k XW1[k, h] * adjT[k, m]; 64 back-to-back
    MMs per half. relu -> bf16, per-128-block xbar-DMA transpose (PE never
    transposes), DMA to DRAM, per-half AllGather (128KB/rank) overlapped
    under the other half's compute / L2 head.
  - Layer 2 re-reads the SBUF-resident slabs (zero adj HBM traffic);
    lhsT k-blocks ordered by availability: own shard straight from SBUF
    (no readback), then AG#0 ranks, then AG#1 ranks.
  - OUT^T = relu(W2^T @ AH^T): 4 MMs + relu, f32 out.
  - XW1 = X @ W1 is row-tiled (K=64 pairs at tile_position (0,0)/(64,0))
    ahead of L1; host packs xT accordingly.
  - Host gathers outT_i ([256, 1024] f32) from each core and transposes.
"""

import numpy as np

N = 8192
D_IN, D_HID, D_OUT = 64, 128, 256
NCORES = 8
SHARD = N // NCORES  # 1024
KB = N // 128  # 64 k-blocks of 128
SLABS = 16  # slab s: k-blocks 8*(s%8)..8*(s%8)+7, m-half s//8
H1S = 64.0  # H1 fp8 pre-scale (1/H1S at relu, H1S folded into w2)


def _build_nc(reps: int = 1, no_adj: bool = False, l1_only: bool = False,
              no_coll: bool = False, no_l2: bool = False, no_out: bool = False):
    from concourse import bacc
    import concourse.mybir as mybir
    import concourse.tile as tile
    from concourse.bass import ts
    from concourse.masks import make_identity

    f32 = mybir.dt.float32
    f32r = mybir.dt.float32r
    bf16 = mybir.dt.bfloat16
    f8 = mybir.dt.float8e4
    Relu = mybir.ActivationFunctionType.Relu

    nc = bacc.Bacc("TRN2", target_bir_lowering=False, debug=False, num_devices=NCORES)

    adjS = nc.dram_tensor("adjS", [SLABS * 128, 8 * 512], f8,
                          kind="ExternalInput").ap()
    xTd = nc.dram_tensor("xTd", [128, N // 2], bf16, kind="ExternalInput").ap()
    w1d = nc.dram_tensor("w1d", [128, D_HID], bf16, kind="ExternalInput").ap()
    w2 = nc.dram_tensor("w2", [D_HID, D_OUT], bf16, kind="ExternalInput").ap()
    outT = nc.dram_tensor("outT", [D_OUT, SHARD], f32, kind="ExternalOutput").ap()

    def body(tc, rep):
        nc = tc.nc

        with (
            tc.tile_pool(name="const", bufs=1) as const_pool,
            tc.tile_pool(name="cache", bufs=1) as cache_pool,
            tc.tile_pool(name="h1p", bufs=1) as h1_pool,
            tc.tile_pool(name="copies", bufs=2) as copy_pool,
            tc.tile_pool(name="dram", bufs=1, space="DRAM") as dram_pool,
        ):
            # ---- ring split: the SYNC ring carries ONLY the slab stream
            # (one HWDGE ring fans out to all 16 SDMA engines, so a single
            # ring sustains full HBM rate); the SCALAR ring carries
            # constants first and is then FREE for the mid-stream H1
            # transpose/write/readback chain — HWDGE rings drain FIFO, so
            # putting that chain on the slab ring would queue it behind
            # ~10us of remaining slabs.
            w1_sb = const_pool.tile([128, D_HID], bf16, name=f"w1sb{rep}")
            nc.scalar.dma_start(w1_sb[:], w1d[:])
            xTd_sb = const_pool.tile([128, N // 2], bf16, name=f"xTsb{rep}")
            for c in range(4):
                nc.scalar.dma_start(xTd_sb[:, ts(c, 1024)], xTd[:, ts(c, 1024)])
            w2_sb = const_pool.tile([D_HID, D_OUT], bf16, name=f"w2sb{rep}")
            nc.scalar.dma_start(w2_sb[:], w2[:])

            # ---- adj slabs: one contiguous 512KB DMA each, all on the
            # sync ring; all 16 stay SBUF-resident for layer 2's re-read.
            # 3D [128, 8, 512]: k-sub-block u at [:, u, :]; layer 2's
            # DoubleRow MMs take pair slices [:, u:u+2, :].
            slabs = []
            for s in range(SLABS):
                st = cache_pool.tile([128, 8, 512], f8,
                                     name=f"slab{rep}_{s}", tag=f"slab{s}")
                if not no_adj:
                    nc.sync.dma_start(st[:], adjS[ts(s, 128), :]
                                      .rearrange("p (k w) -> p k w", k=8))
                slabs.append(st)

            # ---- XW1 = X @ W1, node-major bf16; K=64 pairs run
            # concurrently in row-groups (0,0)/(64,0). The two row-group
            # MMs MUST land in different PSUM banks (same-bank row-split
            # pairs hang the device), so even node-blocks accumulate in
            # psA / odd in psB, and xw1_all is laid out
            # [even blocks 0..4095 | odd blocks 4096..8191].
            xw1_all = const_pool.tile([128, N], bf16, name=f"xw1{rep}")

            def xw1_col(j):  # column of node-block j's lhsT in xw1_all
                return (j // 2) * 128 + (N // 2 if j % 2 else 0)

            with tc.tile_pool(name="xw1_ps", bufs=2, space="PSUM") as xw1_ps_pool:
                for g in range(8):  # group g = pairs 4g..4g+3
                    psA = xw1_ps_pool.tile([128, 512], f32,
                                           name=f"xw1psA{rep}_{g}", tag="xw1psA")
                    psB = xw1_ps_pool.tile([128, 512], f32,
                                           name=f"xw1psB{rep}_{g}", tag="xw1psB")
                    for v in range(4):  # pair u computes blocks (2u, 2u+1)
                        u = 4 * g + v
                        nc.tensor.matmul(psA[:, ts(v, 128)],
                                         xTd_sb[0:64, ts(u, 128)],
                                         w1_sb[0:64, :], start=True, stop=True)
                        nc.tensor.matmul(psB[:, ts(v, 128)],
                                         xTd_sb[64:128, ts(u, 128)],
                                         w1_sb[64:128, :], start=True, stop=True)
                    nc.vector.tensor_copy(xw1_all[:, ts(g, 512)], psA[:])
                    nc.vector.tensor_copy(xw1_all[:, N // 2 + 512 * g:
                                                  N // 2 + 512 * (g + 1)], psB[:])

            # ---- identity for PE-mode transpose
            ident_f32 = const_pool.tile([128, 128], f32, name=f"identf{rep}")
            make_identity(nc, ident_f32[:])
            identity = const_pool.tile([128, 128], f32r, name=f"ident{rep}")
            nc.vector.tensor_copy(identity[:], ident_f32[:])

            # ---- layer 1 + per-half transpose/AllGather/readback.
            # h1c[(half, r)] [128, 512] = rank r's 4 k-blocks 8r+4*half+blk
            h1t = []      # own H1^T halves, node-major [128, 512]: cols blk*128+h
            h1c = {}
            with tc.tile_pool(name="l1_ps", bufs=1, space="PSUM") as l1_ps_pool:
                psum_h = l1_ps_pool.tile([D_HID, SHARD], f32, name=f"l1ps{rep}")
                for half in range(2):
                    for j in range(KB):
                        s = 8 * half + j // 8
                        u = j % 8
                        nc.tensor.matmul(
                            psum_h[:, ts(half, 512)],
                            xw1_all[:, xw1_col(j):xw1_col(j) + 128],
                            slabs[s][:, u, :],
                            start=(j == 0),
                            stop=(j == KB - 1),
                        )
                    # relu with the fp8 pre-scale (1/H1S) folded in; the
                    # inverse rides on w2 (host-scaled by H1S).
                    h1r = copy_pool.tile([D_HID, 512], f32r,
                                         name=f"h1r{rep}_{half}", tag="h1r")
                    nc.scalar.activation(h1r[:], psum_h[:, ts(half, 512)], Relu,
                                         scale=1.0 / H1S)
                    ht = h1_pool.tile([128, 512], f8, name=f"h1t{rep}_{half}",
                                      tag=f"h1t{half}")
                    with tc.tile_pool(name=f"tr_ps{half}", bufs=2,
                                      space="PSUM") as tr_ps_pool:
                        for blk in range(4):
                            tps = tr_ps_pool.tile([128, D_HID], f32r,
                                                  name=f"tps{rep}_{half}_{blk}",
                                                  tag="tps")
                            nc.tensor.transpose(tps[:], h1r[:, ts(blk, 128)],
                                                identity[:])
                            nc.vector.tensor_copy(ht[:, ts(blk, 128)], tps[:])
                    h1t.append(ht)
                    h1_own_dram = dram_pool.tile([128, 512], f8,
                                                 name=f"h1own{rep}_{half}")
                    nc.scalar.dma_start(h1_own_dram[:, :], ht[:])
                    h1_all_dram = dram_pool.tile([NCORES * 128, 512], f8,
                                                 addr_space="Shared",
                                                 name=f"h1all{rep}_{half}")
                    if not no_coll:
                        nc.gpsimd.collective_compute(
                            "AllGather",
                            mybir.AluOpType.bypass,
                            replica_groups=[list(range(NCORES))],
                            ins=[h1_own_dram.opt()],
                            outs=[h1_all_dram.opt()],
                        )
                    if not l1_only:
                        for r in range(NCORES):
                            t = h1_pool.tile([128, 4, 128], f8,
                                             name=f"h1c{rep}_{half}_{r}",
                                             tag=f"h1c{half}_{r}")
                            src = (h1_own_dram[:, :] if no_coll
                                   else h1_all_dram[ts(r, 128), :])
                            nc.scalar.dma_start(
                                t[:], src.rearrange("p (k w) -> p k w", k=4))
                            h1c[(half, r)] = t

            if l1_only:
                nc.sync.dma_start(outT[ts(0, 128), 0:128], h1t[0][:].bitcast(f32))
                return

            # ---- layer 2: psum_ah[h, m] += H1[k, h] * adjT[k, m]; slabs
            # straight from SBUF, fp8 DoubleRow (2 k-blocks per MM).
            # Pairs ordered by availability: AG#0 ranks then AG#1 ranks.
            pairs = [(hs, p, r) for hs in range(2) for p in range(2)
                     for r in range(NCORES)]
            with tc.tile_pool(name="l2_ps", bufs=1, space="PSUM") as l2_ps_pool:
                psum_ah = l2_ps_pool.tile([D_HID, SHARD], f32, name=f"l2ps{rep}")
                ah_sb = copy_pool.tile([D_HID, SHARD], bf16, name=f"ahsb{rep}",
                                       tag="ahsb", bufs=1)
                for half in range(2):
                    if not no_l2:
                        for idx, (hs, p, r) in enumerate(pairs):
                            u = 4 * hs + 2 * p  # first k-sub-block in slab r
                            nc.tensor.matmul(
                                psum_ah[:, ts(half, 512)],
                                h1c[(hs, r)][:, 2 * p:2 * p + 2, :],
                                slabs[8 * half + r][:, u:u + 2, :],
                                start=(idx == 0),
                                stop=(idx == len(pairs) - 1),
                                perf_mode=mybir.MatmulPerfMode.DoubleRow,
                            )
                        nc.vector.tensor_copy(ah_sb[:, ts(half, 512)],
                                              psum_ah[:, ts(half, 512)])
            if no_l2 or no_out:
                nc.sync.dma_start(outT[ts(0, 128), 0:128], h1t[0][:].bitcast(f32))
                return

            # ---- OUT^T = relu(W2^T @ AH^T); hm-major MM order hides the
            # ah half-1 DVE copy, per-half relu+DMA shrinks the tail.
            with tc.tile_pool(name="of_ps", bufs=2, space="PSUM") as of_ps_pool:
                psum_of = [of_ps_pool.tile([128, SHARD], f32,
                                           name=f"ofps{rep}_{ch}", tag=f"ofps{ch}")
                           for ch in range(D_OUT // 128)]
                o_sb = [copy_pool.tile([128, SHARD], f32, name=f"osb{rep}_{ch}",
                                       tag=f"osb{ch}", bufs=1)
                        for ch in range(D_OUT // 128)]
                for hm in range(2):
                    for ch in range(D_OUT // 128):
                        nc.tensor.matmul(psum_of[ch][:, ts(hm, 512)],
                                         w2_sb[:, ts(ch, 128)],
                                         ah_sb[:, ts(hm, 512)],
                                         start=True, stop=True)
                        nc.scalar.activation(o_sb[ch][:, ts(hm, 512)],
                                             psum_of[ch][:, ts(hm, 512)], Relu)
                        nc.sync.dma_start(outT[ts(ch, 128), ts(hm, 512)],
                                          o_sb[ch][:, ts(hm, 512)])

    with tile.TileContext(nc) as tc:
        for rep in range(reps):
            body(tc, rep)
    nc.compile()
    return nc


_NC_CACHE = {}


def get_nc(reps: int = 1, **opts):
    key = (reps, tuple(sorted(opts.items())))
    if key not in _NC_CACHE:
        _NC_CACHE[key] = _build_nc(reps, **opts)
    return _NC_CACHE[key]


def make_in_maps(adj_matrix, node_embs, W1, W2):
    import ml_dtypes

    bf16 = ml_dtypes.bfloat16
    f8 = ml_dtypes.float8_e4m3
    adj_matrix = np.asarray(adj_matrix, dtype=np.float32)
    xT = np.asarray(node_embs, dtype=np.float32).T  # [64, 8192]
    # row-tile pair packing: xTd[0:64, u*128+c] = xT[:, (2u)*128+c],
    #                        xTd[64:128, u*128+c] = xT[:, (2u+1)*128+c]
    xr = xT.reshape(D_IN, N // 256, 2, 128)
    xTd = np.concatenate(
        [xr[:, :, 0, :].reshape(D_IN, N // 2), xr[:, :, 1, :].reshape(D_IN, N // 2)],
        axis=0).astype(bf16)
    w1d = np.concatenate([np.asarray(W1, np.float32)] * 2, axis=0).astype(bf16)
    # H1 is AllGather'd as fp8 scaled by 1/H1S; fold the H1S back into w2
    w2 = (np.asarray(W2, dtype=np.float32) * H1S).astype(bf16)
    in_maps = []
    for i in range(NCORES):
        # adjT_i[k, m] = adj[i*SHARD + m, k]; slab (half*8+s) packs
        # adjS[(half*8+s)*128 + kk, u*512 + m] = adjT_i[(8s+u)*128+kk, half*512+m]
        adjT_i = adj_matrix[i * SHARD:(i + 1) * SHARD, :].T
        adjS = (adjT_i.reshape(8, 8, 128, 2, 512)
                .transpose(3, 0, 2, 1, 4)
                .astype(f8)
                .reshape(SLABS * 128, 8 * 512))
        in_maps.append({"adjS": adjS, "xTd": xTd, "w1d": w1d, "w2": w2})
    return in_maps


def kernel(adj_matrix, node_embs, W1, W2):
    import concourse.bass_utils as bass_utils

    nc = get_nc(reps=1)
    in_maps = make_in_maps(adj_matrix, node_embs, W1, W2)
    res = bass_utils.run_bass_kernel_spmd(nc, in_maps, core_ids=list(range(NCORES)))
    out = np.concatenate([r["outT"].T for r in res.results], axis=0)
    return np.ascontiguousarray(out, dtype=np.float32)


if __name__ == "__main__":
    rng = np.random.default_rng(0)
    adj = rng.random((N, N), dtype=np.float32)
    x = rng.standard_normal((N, D_IN)).astype(np.float32)
    W1 = (rng.standard_normal((D_IN, D_HID)) / np.sqrt(D_IN)).astype(np.float32)
    W2 = (rng.standard_normal((D_HID, D_OUT)) / np.sqrt(D_HID)).astype(np.float32)
    out = kernel(adj_matrix=adj, node_embs=x, W1=W1, W2=W2)
    h = np.maximum(adj @ (x @ W1), 0)
    expected = np.maximum(adj @ (h @ W2), 0)
    err = np.abs(out - expected).max() / np.abs(expected).max()
    print("rel err vs numpy:", err)
